# revision 23
# baseline (speedup 1.0000x reference)
"""Trainium2 Bass kernel for nn_ContrastiveMSELoss (8192x8192 cos-sim contrastive + MSE).

Sharding: 8 NeuronCores, users row-sharded 1024/core, full recipe table per core.

The loss decomposes so the 8192x8192 ratings matrix is never materialized:
    rowR[i]  = 0.1*M + sum_{final scatter cells in row i}(v - 0.1)
    S1       = 0.1*T + sum_pairs (v-0.1)*cos[u,i],  T = (sum_i u_i/|u_i|) . (sum_j r_j/|r_j|)
    S2       = sum_i rowR[i] * log(rowsum_exp[i])
    S3       = sum_i rowR[i] * log(colsum_exp[i])    (col_sum indexed by i: torch n==m quirk)
    loss     = 0.5*(S2 + S3 - 2*S1)/(2*N) + 0.5*mean((ratings-cos_sim)^2)

v2: pair rows are host-gathered (bf16, u||r) instead of device dma_gather; the
row softmax sum rides the EXP activation's accum_out; ACT table loads batched
(Ln then Exp once each, tail Lns together); R transposed in bf16; column-sum
PSUM accumulators DMA straight to DRAM for the ReduceScatter. Host does index
prep (dedup last-write-wins, bincount, sharding) and sums 8x5 partial scalars.
"""

import sys

sys.path.insert(0, "/opt/trn_rl_repo")

import numpy as np

import concourse.bass as bass
import concourse.bacc as bacc
import concourse.tile as tile
from concourse import mybir
from concourse.bass_utils import run_bass_kernel_spmd
from concourse.masks import make_identity

f32 = mybir.dt.float32
bf16 = mybir.dt.bfloat16
AF = mybir.ActivationFunctionType
OP = mybir.AluOpType
AX = mybir.AxisListType

NCORES = 8
N = 8192          # users
M = 8192          # recipes
D = 64
B = 65536
S = N // NCORES   # slab rows per core (1024)
RT = S // 128     # row tiles per slab (8)
NG = 8            # column groups of 1024
ALPHA = 0.5
FILL = 0.1


def build_nc(K):
    """Build the SPMD Bass program. K = pairs per partition (128*K pair slots/core)."""
    nc = bacc.Bacc(num_devices=NCORES)

    u_slab = nc.declare_dram_parameter("u_slab", [S, D], f32, isOutput=False)
    r_full = nc.declare_dram_parameter("r_full", [M, D], f32, isOutput=False)
    row_r_slab = nc.declare_dram_parameter("row_r_slab", [S], f32, isOutput=False)
    row_r_s3 = nc.declare_dram_parameter("row_r_s3", [S], f32, isOutput=False)
    pairs_d = nc.declare_dram_parameter("pairs", [128, K * 128], bf16, isOutput=False)
    pair_w = nc.declare_dram_parameter("pair_w", [128, K], f32, isOutput=False)
    mse_ab = nc.declare_dram_parameter("mse_ab", [2 * (B // NCORES)], f32, isOutput=False)
    out_d = nc.declare_dram_parameter("out", [1, 8], f32, isOutput=True)

    with tile.TileContext(nc) as tc:
        with tc.tile_pool(name="sb", bufs=1) as sb, \
             tc.tile_pool(name="dram", bufs=1, space="DRAM") as dpool:
            # ---- constants ----
            ident_bf = sb.tile([128, 128], bf16)
            make_identity(nc, ident_bf[:])
            ones_bf = sb.tile([128, 1], bf16)
            nc.vector.memset(ones_bf[:], 1.0)
            ones_f = sb.tile([128, 1], f32)
            nc.vector.memset(ones_f[:], 1.0)

            # ---- input loads ----
            u_raw = sb.tile([128, RT, D], f32)   # user r*128+p -> [p, r, :]
            nc.sync.dma_start(out=u_raw[:], in_=u_slab[:].rearrange("(r p) d -> p r d", p=128))
            # recipes p-contiguous: row p*64+n -> [p, n, :]; 16KB/partition
            # descriptors, loaded in halves so normalize pipelines with the DMA
            r_raw = sb.tile([128, 64, D], f32)
            r_src = r_full[:].rearrange("(p n) d -> p n d", p=128)
            for q in range(4):
                qs = slice(16 * q, 16 * (q + 1))
                nc.sync.dma_start(out=r_raw[:, qs, :], in_=r_src[:, qs, :])
            Pg = sb.tile([128, K, 128], bf16)    # pair slot (p,k): [0:64]=U row, [64:128]=R row
            nc.sync.dma_start(out=Pg[:], in_=pairs_d[:].rearrange("p (k d) -> p k d", d=128))
            pw = sb.tile([128, K], f32)
            nc.sync.dma_start(out=pw[:], in_=pair_w[:])
            rowr_sb = sb.tile([128, RT], f32)
            nc.sync.dma_start(out=rowr_sb[:], in_=row_r_slab[:].rearrange("(r p) -> p r", p=128))
            rowr3_sb = sb.tile([128, RT], f32)
            nc.sync.dma_start(out=rowr3_sb[:], in_=row_r_s3[:].rearrange("(r p) -> p r", p=128))
            mab = sb.tile([128, 128], f32)
            nc.sync.dma_start(out=mab[:], in_=mse_ab[:].rearrange("(p k) -> p k", p=128))

            cc_in = dpool.tile([M], f32)
            cc_out = dpool.tile([S], f32)

            with tc.tile_pool(name="psM", bufs=1, space="PSUM") as psM:
                # ---- norms: squares + reduce + reciprocal (DVE), Sqrt (ACT,
                # one table for u and both r halves -> no table thrash) ----
                usq = sb.tile([128, RT, D], f32)
                nc.vector.tensor_tensor(out=usq[:], in0=u_raw[:], in1=u_raw[:], op=OP.mult)
                ssq_u = sb.tile([128, RT], f32)
                nc.vector.tensor_reduce(out=ssq_u[:], in_=usq[:], axis=AX.X, op=OP.add)
                rec_u = sb.tile([128, RT], f32)
                nc.vector.reciprocal(out=rec_u[:], in_=ssq_u[:])
                invu = sb.tile([128, RT], f32)
                nc.scalar.activation(out=invu[:], in_=rec_u[:], func=AF.Sqrt)

                rsq = sb.tile([128, 64, D], f32)
                ssq_r = sb.tile([128, 64], f32)
                rec_r = sb.tile([128, 64], f32)
                invr = sb.tile([128, 64], f32)
                rhat = sb.tile([128, 64, D], bf16)
                for q in range(4):
                    qs = slice(16 * q, 16 * (q + 1))
                    nc.vector.tensor_tensor(
                        out=rsq[:, qs, :], in0=r_raw[:, qs, :], in1=r_raw[:, qs, :], op=OP.mult)
                    nc.vector.tensor_reduce(
                        out=ssq_r[:, qs], in_=rsq[:, qs, :], axis=AX.X, op=OP.add)
                    nc.vector.reciprocal(out=rec_r[:, qs], in_=ssq_r[:, qs])
                    nc.scalar.activation(out=invr[:, qs], in_=rec_r[:, qs], func=AF.Sqrt)
                    nc.vector.tensor_tensor(
                        out=rhat[:, qs, :], in0=r_raw[:, qs, :],
                        in1=invr[:, qs][:, :, None].to_broadcast([128, 16, D]), op=OP.mult)
                u_bf = sb.tile([128, RT, D], bf16)
                nc.vector.tensor_copy(out=u_bf[:], in_=u_raw[:])

                # ---- transposes: U then R per group, shared psum tag ----
                UT = sb.tile([64, S], bf16)
                ptu = psM.tile([64, 1024], bf16, tag="tr", bufs=2)
                for r in range(RT):
                    nc.tensor.transpose(
                        out=ptu[:, r * 128:(r + 1) * 128], in_=u_bf[:, r, :],
                        identity=ident_bf[:])
                nc.vector.tensor_copy(out=UT[:], in_=ptu[:])

                RT_sb = sb.tile([64, M], bf16)
                sr_parts = sb.tile([64, NG], f32)

                def phase_a(g):
                    ptr = psM.tile([64, 1024], bf16, tag="tr", bufs=2)
                    for t in range(8):
                        nc.tensor.transpose(
                            out=ptr[:, t * 128:(t + 1) * 128], in_=rhat[:, g * 8 + t, :],
                            identity=ident_bf[:])
                    nc.vector.tensor_scalar(
                        out=RT_sb[:, g * 1024:(g + 1) * 1024], in0=ptr[:],
                        scalar1=1.0, scalar2=None, op0=OP.mult, op1=OP.add,
                        accum_out=sr_parts[:, g:g + 1])

                # ---- main loop state ----
                rs_parts = sb.tile([128, RT * NG], f32)

                def phase_b(g):
                    cs_g = psM.tile([128, 512], f32, tag="cs", bufs=2, name=f"cs{g}")
                    for r in range(RT):
                        pg = psM.tile([128, 1024], f32, tag="cos", bufs=2)
                        for jj in range(2):
                            j = g * 2 + jj
                            nc.tensor.matmul(
                                out=pg[:, jj * 512:(jj + 1) * 512],
                                lhsT=UT[:, r * 128:(r + 1) * 128],
                                rhs=RT_sb[:, j * 512:(j + 1) * 512],
                                start=True, stop=True)
                        ex = sb.tile([128, 1024], bf16, tag="exp", bufs=4)
                        nc.scalar.activation(
                            out=ex[:], in_=pg[:], func=AF.Exp, scale=invu[:, r:r + 1],
                            accum_out=rs_parts[:, r * NG + g:r * NG + g + 1])
                        for jj in range(2):
                            nc.tensor.matmul(
                                out=cs_g[32 * jj:32 * jj + 1, :],
                                lhsT=ones_bf[:, 0:1],
                                rhs=ex[:, jj * 512:(jj + 1) * 512],
                                start=(r == 0), stop=(r == RT - 1),
                                tile_position=(0, 32 * jj),
                                skip_group_check=True)
                    # colsum partials for blocks 2g, 2g+1 -> DRAM (progressive);
                    # PSUM->SBUF bounce (GpSimd cannot read PSUM, DVE is idle here)
                    csb = sb.tile([128, 2, 512], f32, tag="csb", bufs=2, name=f"csb{g}")
                    nc.vector.tensor_copy(out=csb[0:1, 0, :], in_=cs_g[0:1, :])
                    nc.vector.tensor_copy(out=csb[32:33, 1, :], in_=cs_g[32:33, :])
                    nc.sync.dma_start(out=cc_in[(2 * g) * 512:(2 * g + 1) * 512], in_=csb[0:1, 0, :])
                    nc.sync.dma_start(out=cc_in[(2 * g + 1) * 512:(2 * g + 2) * 512], in_=csb[32:33, 1, :])

                for g in range(NG):
                    phase_a(g)

                # staged ReduceScatter: early stages fire mid-phase_b so the
                # collective latency hides under the remaining column groups;
                # the last stage only carries 1024 columns (4KB)
                def rs_stage(c0, c1):
                    nc.gpsimd.collective_compute(
                        "ReduceScatter", OP.add,
                        replica_groups=[list(range(NCORES))],
                        ins=[cc_in[c0 * 512:c1 * 512].opt()],
                        outs=[cc_out[(c0 * 512) // NCORES:(c1 * 512) // NCORES].opt()])

                stages = {3: (0, 8), 5: (8, 12), 6: (12, 14), 7: (14, 16)}
                for g in range(NG):
                    phase_b(g)
                    if g in stages:
                        rs_stage(*stages[g])

            # ---- pair math on DVE: covers the tail collective latency ----
            sq = sb.tile([128, K, 128], bf16)
            nc.vector.tensor_tensor(out=sq[:], in0=Pg[:], in1=Pg[:], op=OP.mult)
            nrm2 = sb.tile([128, 2 * K], f32)
            nc.vector.tensor_reduce(
                out=nrm2[:], in_=sq[:].rearrange("p k (h d) -> p (k h) d", h=2),
                axis=AX.X, op=OP.add)
            n2 = sb.tile([128, K], f32)
            nc.vector.tensor_reduce(
                out=n2[:], in_=nrm2[:].rearrange("p (k h) -> p k h", h=2),
                axis=AX.X, op=OP.mult)
            prod = sb.tile([128, K, D], bf16)
            nc.vector.tensor_tensor(
                out=prod[:], in0=Pg[:, :, 0:64], in1=Pg[:, :, 64:128], op=OP.mult)
            dot = sb.tile([128, K], f32)
            nc.vector.tensor_reduce(out=dot[:], in_=prod[:], axis=AX.X, op=OP.add)

            # =============== tail (overlaps the collective) ===============
            with tc.tile_pool(name="psT", bufs=1, space="PSUM") as psT:
                # T partial: sum_{p,r} invu * (u_raw . sR)
                sr_f = sb.tile([64, 1], f32)
                nc.vector.tensor_reduce(out=sr_f[:], in_=sr_parts[:], axis=AX.X, op=OP.add)
                sr_bf = sb.tile([64, 1], bf16)
                nc.vector.tensor_copy(out=sr_bf[:], in_=sr_f[:])
                psTT = psT.tile([128, RT], f32)
                for r in range(RT):
                    nc.tensor.matmul(
                        out=psTT[:, r:r + 1], lhsT=UT[:, r * 128:(r + 1) * 128],
                        rhs=sr_bf[:], start=True, stop=True)
                tdot = sb.tile([128, RT], f32)
                nc.vector.tensor_copy(out=tdot[:], in_=psTT[:])
                tw = sb.tile([128, RT], f32)
                nc.vector.tensor_tensor(out=tw[:], in0=tdot[:], in1=invu[:], op=OP.mult)
                t_acc = sb.tile([128, 1], f32)
                nc.vector.tensor_reduce(out=t_acc[:], in_=tw[:], axis=AX.X, op=OP.add)

                # S2: sum rowR_slab * ln(rowsum)   [ACT: Ln batch starts here]
                rs_r = sb.tile([128, RT], f32)
                nc.vector.tensor_reduce(
                    out=rs_r[:], in_=rs_parts[:].rearrange("p (r g) -> p r g", g=NG),
                    axis=AX.X, op=OP.add)
                lrs = sb.tile([128, RT], f32)
                nc.scalar.activation(out=lrs[:], in_=rs_r[:], func=AF.Ln)
                s2w = sb.tile([128, RT], f32)
                nc.vector.tensor_tensor(out=s2w[:], in0=lrs[:], in1=rowr_sb[:], op=OP.mult)
                s2_acc = sb.tile([128, 1], f32)
                nc.vector.tensor_reduce(out=s2_acc[:], in_=s2w[:], axis=AX.X, op=OP.add)

                # pair cos: ln(n2) -> later exp(-0.5*ln) after the other Lns
                ln2 = sb.tile([128, K], f32)
                nc.scalar.activation(out=ln2[:], in_=n2[:], func=AF.Ln)

                # S3 (after ReduceScatter lands): sum rowR_slab * ln(colsum_slab)
                lcs_in = sb.tile([128, RT], f32)
                nc.sync.dma_start(out=lcs_in[:], in_=cc_out[:].rearrange("(r p) -> p r", p=128))
                lcs = sb.tile([128, RT], f32)
                nc.scalar.activation(out=lcs[:], in_=lcs_in[:], func=AF.Ln)
                s3w = sb.tile([128, RT], f32)
                nc.vector.tensor_tensor(out=s3w[:], in0=lcs[:], in1=rowr3_sb[:], op=OP.mult)
                s3_acc = sb.tile([128, 1], f32)
                nc.vector.tensor_reduce(out=s3_acc[:], in_=s3w[:], axis=AX.X, op=OP.add)

                # pair term: cos = dot * rsqrt(u2*r2), weighted by w
                inv = sb.tile([128, K], f32)
                nc.scalar.activation(out=inv[:], in_=ln2[:], func=AF.Exp, scale=-0.5)
                cosg = sb.tile([128, K], f32)
                nc.vector.tensor_tensor(out=cosg[:], in0=dot[:], in1=inv[:], op=OP.mult)
                cosgw = sb.tile([128, K], f32)
                nc.vector.tensor_tensor(out=cosgw[:], in0=cosg[:], in1=pw[:], op=OP.mult)
                w_acc = sb.tile([128, 1], f32)
                nc.vector.tensor_reduce(out=w_acc[:], in_=cosgw[:], axis=AX.X, op=OP.add)

                # MSE
                md = sb.tile([128, 64], f32)
                nc.vector.tensor_tensor(out=md[:], in0=mab[:, 0:64], in1=mab[:, 64:128], op=OP.subtract)
                msq = sb.tile([128, 64], f32)
                nc.vector.tensor_tensor(out=msq[:], in0=md[:], in1=md[:], op=OP.mult)
                m_acc = sb.tile([128, 1], f32)
                nc.vector.tensor_reduce(out=m_acc[:], in_=msq[:], axis=AX.X, op=OP.add)

                # partition-reduce the five partials via ones-matmul
                combo = sb.tile([128, 5], f32)
                nc.vector.tensor_copy(out=combo[:, 0:1], in_=s2_acc[:])
                nc.vector.tensor_copy(out=combo[:, 1:2], in_=s3_acc[:])
                nc.vector.tensor_copy(out=combo[:, 2:3], in_=t_acc[:])
                nc.vector.tensor_copy(out=combo[:, 3:4], in_=w_acc[:])
                nc.vector.tensor_copy(out=combo[:, 4:5], in_=m_acc[:])
                po = psT.tile([1, 5], f32)
                nc.tensor.matmul(out=po[:], lhsT=ones_f[:, 0:1], rhs=combo[:], start=True, stop=True)
                out_sb = sb.tile([1, 8], f32)
                nc.vector.memset(out_sb[:], 0.0)
                nc.vector.tensor_copy(out=out_sb[:, 0:5], in_=po[:])
                nc.sync.dma_start(out=out_d[:], in_=out_sb[:])
    nc.finalize()
    return nc


def _host_prep(inputs):
    """Dedup scatter (last write wins), shard pairs by row slab, build per-core arrays."""
    U = np.ascontiguousarray(np.asarray(inputs["user_embeddings"], dtype=np.float32))
    R = np.ascontiguousarray(np.asarray(inputs["recipe_embeddings"], dtype=np.float32))
    rat = np.asarray(inputs["ratings_scaled"], dtype=np.float32)
    css = np.asarray(inputs["cos_similarities_scaled"], dtype=np.float32)
    u = np.asarray(inputs["u_idx"]).astype(np.int64)
    i = np.asarray(inputs["i_idx"]).astype(np.int64)

    cell = u * M + i
    _, idx_rev = np.unique(cell[::-1], return_index=True)
    keep = (B - 1 - idx_rev)  # last occurrences, sorted by cell (=> sorted by u)
    uu = u[keep].astype(np.int32)
    ii = i[keep].astype(np.int32)
    ww = (rat[keep] - FILL).astype(np.float32)

    delta = np.bincount(uu, weights=ww.astype(np.float64), minlength=N)
    row_r = (FILL * M + delta).astype(np.float32)

    core_of = uu // S
    counts = np.bincount(core_of, minlength=NCORES)
    K = int(np.ceil(counts.max() / 128))
    cap = 128 * K

    import ml_dtypes
    Ub = U.astype(ml_dtypes.bfloat16)
    Rb = R.astype(ml_dtypes.bfloat16)

    # colsum column c holds recipe row (c%128)*64 + c//128 (p-contiguous load
    # + PE transpose). Staged ReduceScatter: within each stage's column range
    # core k owns the k-th contiguous 1/8 share.
    col_r = np.empty((NCORES, S), dtype=np.float32)
    for c in range(NCORES):
        cols = np.concatenate([
            c0 * 512 + c * (c1 - c0) * 64 + np.arange((c1 - c0) * 64)
            for c0, c1 in [(0, 8), (8, 12), (12, 14), (14, 16)]])
        col_r[c] = row_r[(cols % 128) * 64 + cols // 128]

    in_maps = []
    bs = B // NCORES
    for c in range(NCORES):
        sel = core_of == c
        n_c = int(sel.sum())
        # pair buffer [128, K, 128]: slot (p, k) = pair k*128+p; [0:64]=U row, [64:128]=R row
        P = np.zeros((cap, 128), dtype=ml_dtypes.bfloat16)
        P[:, 0] = 1.0   # padding rows: unit basis vectors in both halves
        P[:, 64] = 1.0  # (norms 1, cos 1, w 0)
        P[:n_c, 0:64] = Ub[uu[sel]]
        P[:n_c, 64:128] = Rb[ii[sel]]
        P = np.ascontiguousarray(
            P.reshape(K, 128, 128).transpose(1, 0, 2).reshape(128, K * 128))
        W = np.zeros((cap,), dtype=np.float32)
        W[:n_c] = ww[sel]
        W = np.ascontiguousarray(W.reshape(K, 128).T)
        in_maps.append({
            "u_slab": np.ascontiguousarray(U[c * S:(c + 1) * S]),
            "r_full": R,
            "row_r_slab": np.ascontiguousarray(row_r[c * S:(c + 1) * S]),
            "row_r_s3": col_r[c],
            "pairs": P,
            "pair_w": W,
            "mse_ab": np.concatenate([
                rat[c * bs:(c + 1) * bs].reshape(128, 64),
                css[c * bs:(c + 1) * bs].reshape(128, 64)], axis=1).ravel(),
        })
    return in_maps, K


def kernel(user_embeddings, recipe_embeddings, ratings_scaled, cos_similarities_scaled,
           u_idx, i_idx, _trace=False):
    inputs = {
        "user_embeddings": user_embeddings,
        "recipe_embeddings": recipe_embeddings,
        "ratings_scaled": ratings_scaled,
        "cos_similarities_scaled": cos_similarities_scaled,
        "u_idx": u_idx,
        "i_idx": i_idx,
    }
    in_maps, K = _host_prep(inputs)
    nc = build_nc(K)
    res = run_bass_kernel_spmd(nc, in_maps, core_ids=list(range(NCORES)), trace=_trace)
    outs = np.stack([res.results[c]["out"][0] for c in range(NCORES)])  # [8, 8]
    o = outs.astype(np.float64)
    S2 = o[:, 0].sum()
    S3 = o[:, 1].sum()
    T = o[:, 2].sum()
    PAIR = o[:, 3].sum()
    MSE_SUM = o[:, 4].sum()
    contrastive = (S2 + S3 - 2.0 * (FILL * T + PAIR)) / (2.0 * N)
    loss = ALPHA * contrastive + (1.0 - ALPHA) * (MSE_SUM / B)
    if _trace:
        kernel._last_results = res
    return np.float32(loss)
